# revision 17
# baseline (speedup 1.0000x reference)
"""Single-head causal attention (B=4, T=4096, E=1024, H=128) on 8 trn2 cores.

Sharding: core c -> (batch b = c//2, piece p = c%2). Within a batch the 32
query blocks of 128 rows are split even/odd between the two pieces so the
causal workload balances. The device program is identical on all cores
(SPMD); all per-core differences are carried by the input data:
  - xT arrives column-PERMUTED per core: within every 256-token block the
    core's own 128 query tokens come first. The Q projection then reads a
    fixed strided slice of the same x tiles used for K/V (no separate
    gathered copy), and the program stays core-independent.
  - the causal-boundary mask strip is per-core data.

Device algorithm (per core, all "transposed" layouts):
  per round tt (1024 permuted tokens = key blocks 8tt..8tt+7):
    KT chunk = Wk @ x^T            [H=128, 512] x2      (PSUM->SBUF on DVE)
    QT tile  = Wq @ xq^T           [H=128, 512]   (strided in-tile gather)
    V blocks = (x_blk^T)^T @ Wv^T  [128 tok, 128 h] direct (no transpose)
    attention for q-tile tt over key-block PAIRS g (kb=2g, 2g+1):
      ST[2g+i] = KT_blk^T @ QT_tile into psum pair slot i   (c0-trimmed)
      boundary blocks get an additive causal mask strip (per-core data)
      ONE exp over the whole pair slab  [128, 2, 512-c0] -> PT f16
      for i in (0,1): OT += V_blk^T @ PT[i]; pacc[i] += PT[i] (DVE)
    lb = allones^T @ (pacc0+pacc1); O = OT * recip(lb) -> out[:, tile]
  projections are software-pipelined into the attention iterations.
Host transposes [H, TQ] -> [TQ, H] when scattering into the full output.
"""

import numpy as np

B, T, E, H = 4, 4096, 1024, 128
P = 128
NB_E = E // P           # 8 contraction chunks
TQ = T // 2             # 2048 gathered queries per core
N_RND = 4               # rounds; round tt = permuted tokens 1024tt..+1023
SCALE = float(H) ** -0.5
NEG = -30000.0
N_CORES = 8
F32 = np.float32


def _query_rows(p: int) -> np.ndarray:
    """Absolute row indices of the gathered queries for piece p (in order)."""
    blocks = [np.arange(256 * g + 128 * p, 256 * g + 128 * p + 128) for g in range(16)]
    return np.concatenate(blocks)


def _perm_cols(p: int) -> np.ndarray:
    """Permuted token order for core piece p: within each 256-token block the
    own 128 tokens (offset 128p) come first, the other 128 after."""
    out = []
    for g in range(16):
        own = np.arange(256 * g + 128 * p, 256 * g + 128 * p + 128)
        oth = np.arange(256 * g + 128 * (1 - p), 256 * g + 128 * (1 - p) + 128)
        out.append(own)
        out.append(oth)
    return np.concatenate(out)


def _mask_strip(p: int) -> np.ndarray:
    """maskT [1024 k, 512 q] f16: 0 where key visible, NEG where masked.

    Row 128*j + kk is PERMUTED in-strip key block j (j=0..7); col 128*i + r
    is in-tile query block i. Permuted block j holds original block
    jp = j + p*(1-2*(j%2)); visible iff 128*jp + kk <= 256*i + 128*p + r.
    """
    j = np.arange(1024)[:, None] // 128
    kk = np.arange(1024)[:, None] % 128
    jp = j + p * (1 - 2 * (j % 2))
    qq = np.arange(512)[None, :]
    i, r = qq // 128, qq % 128
    visible = 128 * jp + kk <= 256 * i + 128 * p + r
    return np.where(visible, 0.0, NEG).astype(np.float16)


def _c0_of(tt: int, kb: int) -> int:
    """First un-skippable query column for key block kb in q-tile tt."""
    if kb < 8 * tt:
        return 0
    j = kb - 8 * tt
    return P * max(0, -(-(128 * j - 255) // 256))


def _emit(tc, aps):
    from concourse import mybir

    nc = tc.nc
    f32 = mybir.dt.float32
    f16 = mybir.dt.float16
    EXP = mybir.ActivationFunctionType.Exp

    from contextlib import ExitStack

    xT, wqkv, maskT, out = aps

    ctx = ExitStack()
    with ctx:
        # ---- pools ----
        consts = ctx.enter_context(tc.tile_pool(name="consts", bufs=1))
        x_pool = ctx.enter_context(tc.tile_pool(name="x", bufs=3))
        qt_pool = ctx.enter_context(tc.tile_pool(name="qt", bufs=2))
        vt_pool = ctx.enter_context(tc.tile_pool(name="vt", bufs=2))
        pt_pool = ctx.enter_context(tc.tile_pool(name="pt", bufs=3))
        pa_pool = ctx.enter_context(tc.tile_pool(name="pa", bufs=4))
        osb_pool = ctx.enter_context(tc.tile_pool(name="osb", bufs=2))
        rl_pool = ctx.enter_context(tc.tile_pool(name="rl", bufs=2))
        # PSUM: score pair slabs 2x2 banks + proj 2 + ot 2 = 8 banks.
        # lb squats in an idle s_ps slot (scores are done when lb runs).
        s_ps = ctx.enter_context(tc.tile_pool(name="sps", bufs=2, space="PSUM"))
        o_ps = ctx.enter_context(tc.tile_pool(name="ops", bufs=2, space="PSUM"))
        p_ps = ctx.enter_context(tc.tile_pool(name="pps", bufs=2, space="PSUM"))

        # ---- persistent SBUF tensors ----
        allones = consts.tile([P, P], f16)
        identity = consts.tile([P, P], f16)
        w_sb = consts.tile([P, NB_E, 3, P], f16)   # [., chunk, (k|v|q), .]
        mask_sb = consts.tile([P, 8, 512], f16)
        kt_all = consts.tile([P, T // P, P], f16)
        v_all = consts.tile([P, T // P, P], f16)

        nc.gpsimd.memset(allones[:], 1.0)
        from concourse.masks import make_identity
        make_identity(nc, identity[:])

        # ---- x round tiles: [128, chunk c, block b, 256] f16 ----
        x_tiles = [x_pool.tile([P, NB_E, 4, 256], f16, tag="x", name=f"x_{tt}")
                   for tt in range(N_RND)]

        def dma_x(tt, halves, split=1, eng=None):
            """Load round tt's 1024 permuted tokens (halves: list of 0/1).
            split>1 slices the load along E-chunks for finer semaphores.
            eng picks the issuing engine queue (parallel descriptor gen)."""
            engs = eng if eng is not None else [nc.sync]
            k = 0
            for h in halves:
                t0 = tt * 1024 + h * 512
                cw = NB_E // split
                for g in range(split):
                    engs[k % len(engs)].dma_start(
                        x_tiles[tt][:, g * cw:(g + 1) * cw, 2 * h:2 * h + 2, :],
                        xT[g * cw * P:(g + 1) * cw * P, t0:t0 + 512].rearrange(
                            "(c p) (b q) -> p c b q", p=P, b=2),
                    )
                    k += 1

        # startup-critical order: weights first (gate all projections), then
        # round-0/1 x striped across engine queues so descriptor issue is
        # parallel (a single queue serializes at ~1.4us per dma_start).
        # nc.tensor is NOT used: a DMA instruction there would block the PE
        # queue ahead of the warmup matmuls.
        for ch in (0, 4):
            nc.sync.dma_start(
                w_sb[:, ch:ch + 4, :, :],
                wqkv[ch * P:(ch + 4) * P, :, :].rearrange(
                    "(c p) s h -> p c s h", p=P))
        dma_x(0, [0], split=4, eng=[nc.scalar, nc.gpsimd])
        dma_x(0, [1], split=2, eng=[nc.scalar, nc.gpsimd])
        for jh in (0, 1):
            nc.sync.dma_start(
                mask_sb[:, 4 * jh:4 * jh + 4, :],
                maskT[512 * jh:512 * jh + 512, :].rearrange(
                    "(j p) q -> p j q", p=P))
        dma_x(1, [0], split=2, eng=[nc.scalar, nc.gpsimd])
        dma_x(1, [1], split=2, eng=[nc.scalar, nc.gpsimd])
        dma_x(2, [0, 1], eng=[nc.sync])

        # PE warmup: dummy matmuls ramp the tensor-engine p-state while the
        # first DMAs stream; their results are never read. Enough to stay
        # busy until the first real matmul's data lands, no more.
        for _ in range(30):
            wp = p_ps.tile([P, P], f32, tag="pps", name="warm")
            nc.tensor.matmul(wp[:], lhsT=allones[:], rhs=allones[:],
                             start=True, stop=True)

        # ---- projection pieces (generators of thunks) ----
        def mk_group(tt, sel, rhs_fn, fin):
            ps = p_ps.tile([P, 512], f32, tag="pps")
            for c in range(NB_E):
                def mm(c=c, ps=ps):
                    nc.tensor.matmul(ps[:], lhsT=w_sb[:, c, sel, :],
                                     rhs=rhs_fn(c), start=(c == 0),
                                     stop=(c == NB_E - 1))
                yield mm
            yield lambda ps=ps: fin(ps)

        def v_group(tt, h):
            """VT = Wv @ x^T (wide matmuls, stationary weights), then PE
            transposes each 128-block into [token, head] layout for PV."""
            xt = x_tiles[tt]
            vt = vt_pool.tile([P, 512], f16, tag="vt", name=f"vt_{tt}_{h}")

            def fin_v(ps, vt=vt):
                nc.vector.tensor_copy(vt[:], ps[:])
            yield from mk_group(
                tt, 1, lambda c, h=h: xt[:, c, 2 * h:2 * h + 2, :], fin_v)
            for u in range(4):
                kb = tt * 8 + 4 * h + u

                def tr(u=u, kb=kb, vt=vt):
                    tp = p_ps.tile([P, P], f16, tag="pps",
                                   name=f"tp_{tt}_{kb}")
                    nc.tensor.transpose(tp[:], vt[:, u * P:(u + 1) * P],
                                        identity[:])
                    nc.vector.tensor_copy(v_all[:, kb, :], tp[:])
                yield tr

        def k_group(tt, h):
            xt = x_tiles[tt]

            def fin_k(ps):
                dst = kt_all[:, tt * 8 + 4 * h: tt * 8 + 4 * h + 4, :]
                nc.vector.tensor_copy(dst, ps[:])
            yield from mk_group(
                tt, 0, lambda c: xt[:, c, 2 * h:2 * h + 2, :], fin_k)

        def q_group(tt, qt):
            xt = x_tiles[tt]

            def fin_q(ps):
                nc.vector.tensor_copy(qt[:], ps[:])
            yield from mk_group(tt, 2, lambda c: xt[:, c, :, 0:128], fin_q)

        def chain(*gens):
            for g in gens:
                yield from g

        def drain(gen, n):
            """Emit up to n pieces; returns False when exhausted."""
            if gen is None:
                return False
            for _ in range(n):
                try:
                    next(gen)()
                except StopIteration:
                    return False
            return True

        qts = [qt_pool.tile([P, 512], f16, tag="qt", name=f"qt_{t}")
               for t in range(N_RND)]

        # round 0 projections run up front (halves as their DMAs land)
        for piece in chain(k_group(0, 0), v_group(0, 0), k_group(0, 1),
                           q_group(0, qts[0])):
            piece()

        # interleave/spill generators:
        #   head(r) = Q + K-A + V-A of round r  (into attention r-1)
        #   tail(r) = K-B + V-B of round r      (may spill into attention r)
        N_HEAD = 9 + 9 + 13   # q + k0 + v0 piece counts
        N_TAIL = 9 + 13       # k1 + v1
        head = lambda r: chain(q_group(r, qts[r]), k_group(r, 0),
                               v_group(r, 0))
        tail = lambda r: chain(k_group(r, 1), v_group(r, 1))

        gen_head = None   # projections of round tt+1, must finish in attn tt
        gen_tail = None   # projections of round tt+1, may spill into attn tt+1
        spill = v_group(0, 1)  # leftovers with deadline in the current round
        n_spill = 13

        for tt in range(N_RND):
            if tt == 1:
                dma_x(3, [0, 1])
            if tt + 1 < N_RND:
                gen_head = chain(head(tt + 1))
                gen_tail = chain(tail(tt + 1))
            else:
                gen_head = gen_tail = None

            nkb = 8 * tt + 8
            npair = nkb // 2
            per_head = -(-N_HEAD // max(1, nkb - 2))

            qs = qts[tt]
            ot = o_ps.tile([P, 512], f32, tag="ops", name=f"ot_{tt}")
            pacc = [pa_pool.tile([P, 512], f16, tag="pa", name=f"pa_{tt}_{i}")
                    for i in range(2)]
            for i in range(2):
                nc.gpsimd.memset(pacc[i][:], 0.0)

            s_tiles = [None] * npair

            def emit_scores(g, tt=tt, qs=qs, s_tiles=s_tiles):
                """Score pair g: kb=2g,2g+1 into one 2-bank psum slab."""
                s = s_ps.tile([P, 2, 512], f32, tag="sps",
                              name=f"s_{tt}_{g}")
                for i in range(2):
                    kb = 2 * g + i
                    c0 = _c0_of(tt, kb)
                    nc.tensor.matmul(
                        s[:, i, c0:512],
                        lhsT=kt_all[:, kb, :],
                        rhs=qs[:, c0:512],
                        start=True, stop=True,
                    )
                s_tiles[g] = s

            emit_scores(0)
            # spill pacing: the spilled tail of THIS round's projections
            # provides kt blocks 8tt+4.. (read at score pair 4tt+1, K pieces
            # come first in the chain) and v blocks 8tt+4..7 (read by PV of
            # pairs 4tt+2..4tt+3). Drain evenly so all land by pair 4tt+3.
            sp_dl = 3 if tt == 0 else 4 * tt + 3
            sp_rate = -(-n_spill // sp_dl) if spill is not None else 0

            for g in range(npair):
                if spill is not None and g < sp_dl:
                    if not drain(spill, sp_rate):
                        spill = None

                if g + 1 < npair:
                    emit_scores(g + 1)
                s = s_tiles[g]
                c0p = _c0_of(tt, 2 * g)      # pair c0 (equal within pair
                for i in range(2):           # for diagonal pairs)
                    kb = 2 * g + i
                    if kb >= 8 * tt:
                        j = kb - 8 * tt
                        c0 = _c0_of(tt, kb)
                        nc.vector.tensor_add(
                            s[:, i, c0:c0 + P], s[:, i, c0:c0 + P],
                            mask_sb[:, j, c0:c0 + P])
                pt = pt_pool.tile([P, 2, 512], f16, tag="pt")
                nc.scalar.activation(pt[:, :, c0p:512], s[:, :, c0p:512],
                                     EXP, scale=SCALE)
                for i in range(2):
                    kb = 2 * g + i
                    c0 = _c0_of(tt, kb)
                    nc.tensor.matmul(
                        ot[:, c0:512],
                        lhsT=v_all[:, kb, :],
                        rhs=pt[:, i, c0:512],
                        start=(kb == 0),
                        stop=(kb == nkb - 1),
                    )
                    nc.vector.tensor_add(pacc[i][:, c0:512],
                                         pacc[i][:, c0:512],
                                         pt[:, i, c0:512])

                if gen_head is not None:
                    if not drain(gen_head, 2 * per_head):
                        gen_head = None
                elif gen_tail is not None and g >= npair - 4:
                    if not drain(gen_tail, 6):
                        gen_tail = None

            # anything left of the head must land now; tail becomes spill
            while drain(gen_head, 4):
                pass
            gen_head = None
            spill = gen_tail
            n_spill = N_TAIL
            gen_tail = None

            # epilogue: softmax denominator + normalize + store (the last
            # tile streams in halves so the store overlaps the arithmetic).
            # lb reuses an idle score-slab psum slot (scores are all read).
            lb = s_ps.tile([P, 512], f32, tag="sps", name=f"lb_{tt}")
            halves = (0, 256) if tt == N_RND - 1 else (0,)
            width = 512 // len(halves)
            for hb in halves:
                sl = slice(hb, hb + width)
                nc.tensor.matmul(lb[:, sl], lhsT=allones[:],
                                 rhs=pacc[0][:, sl], start=True, stop=False)
                nc.tensor.matmul(lb[:, sl], lhsT=allones[:],
                                 rhs=pacc[1][:, sl], start=False, stop=True)
                rl = rl_pool.tile([P, width], f32, tag="rl")
                nc.vector.reciprocal_approx_fast(rl[:], lb[:, sl])
                o_sb = osb_pool.tile([P, width], f32, tag="osb")
                nc.vector.tensor_mul(o_sb[:], ot[:, sl], rl[:])
                nc.gpsimd.dma_start(
                    out[:, tt * 512 + hb: tt * 512 + hb + width], o_sb[:])


def build_program():
    import concourse.tile as tile
    from concourse import bacc, mybir

    f32 = mybir.dt.float32
    f16 = mybir.dt.float16
    nc = bacc.Bacc("TRN2", target_bir_lowering=False, debug=False,
                   num_devices=N_CORES)
    xT = nc.dram_tensor("xT", [E, T], f16, kind="ExternalInput").ap()
    wqkv = nc.dram_tensor("wqkv", [E, 3, H], f16, kind="ExternalInput").ap()
    maskT = nc.dram_tensor("maskT", [1024, 512], f16, kind="ExternalInput").ap()
    out = nc.dram_tensor("out", [H, TQ], f32, kind="ExternalOutput").ap()

    with tile.TileContext(nc) as tc:
        _emit(tc, (xT, wqkv, maskT, out))
    nc.compile()
    return nc


def make_in_maps(x, Wq, Wk, Wv):
    """Per-core input maps. x: [B,T,E] f32; W*: [H,E] f32."""
    x = np.asarray(x, dtype=F32)
    # combined [E, 3, H] with slot order (k, v, q)
    wqkv = np.stack(
        [np.asarray(Wk, F32).T, np.asarray(Wv, F32).T, np.asarray(Wq, F32).T],
        axis=1).astype(np.float16)
    wqkv = np.ascontiguousarray(wqkv)
    masks = [_mask_strip(0), _mask_strip(1)]
    perms = [_perm_cols(0), _perm_cols(1)]
    in_maps = []
    for c in range(N_CORES):
        b, p = c // 2, c % 2
        xb = x[b][perms[p]]                                    # [T, E] permuted
        xT_np = np.ascontiguousarray(xb.T.astype(np.float16))
        in_maps.append({
            "xT": xT_np,
            "wqkv": wqkv,
            "maskT": masks[p],
        })
    return in_maps


def run(x, Wq, Wk, Wv, trace=False, trace_cores=None):
    """Returns (full_output [B,T,H] f32, BassKernelResults)."""
    from concourse.bass_utils import run_bass_kernel_spmd

    nc = build_program()
    in_maps = make_in_maps(x, Wq, Wk, Wv)
    res = run_bass_kernel_spmd(
        nc, in_maps, list(range(N_CORES)), trace=trace,
        trace_cores=trace_cores,
    )
    full = np.empty((B, T, H), dtype=F32)
    for c in range(N_CORES):
        b, p = c // 2, c % 2
        full[b, _query_rows(p), :] = res.results[c]["out"].T
    return full, res


def kernel(x, Wq, Wk, Wv):
    full, _ = run(x, Wq, Wk, Wv, trace=False)
    return full


if __name__ == "__main__":
    nc = build_program()
    print("program built ok")


# revision 18
# speedup vs baseline: 1.1635x; 1.1635x over previous
"""Single-head causal attention (B=4, T=4096, E=1024, H=128) on 8 trn2 cores.

Sharding (key-split): core c -> (batch b = c//2, piece p = c%2). The two
cores of a batch split the KEYS: within every 256-token block, piece p owns
the 128 tokens at offset 128p. Each core:
  - projects K/V only for its OWN 2048 keys (no duplicate K/V compute),
  - projects Q for ALL 4096 queries,
  - computes the partial softmax numerator o = sum_own exp(s) v and
    denominator l = sum_own exp(s) over its own keys only.
The host merges: out = (o0 + o1) / (l0 + l1) per batch. This trades a
duplicated Q projection (cheap) for the K/V projection duplication
(expensive) of a query-split, cutting tensor-engine work ~9%.

SPMD trick: xT arrives column-PERMUTED per core (own 128 first within each
256-block), so "own keys" sit at fixed in-tile offsets and the device
program is identical on all cores; the causal boundary mask strip is
per-core data. Queries stay in permuted order end-to-end; the host
unpermutes when scattering (the permutation is an involution).

Device algorithm (per core, transposed layouts):
  per round tt = 0..7 (q-tile = permuted query cols 512tt..+511):
    project (during attention of round tt-1):
      QT tile  = Wq @ x^T          [H, 512]  (contiguous cols)
      KT own   = Wk @ x_own^T      [H, 2, 128] -> kt blocks 2tt, 2tt+1
      VT own   = Wv @ x_own^T -> f16 -> PE-transpose -> V blocks [128t,128h]
    attention over own-key PAIRS g=0..tt (pair g = own blocks 2g, 2g+1):
      ST[i] = KT_blk^T @ QT_tile  [128k, 512q] into a 2-bank psum slab
      diagonal pair (g==tt) gets additive mask strips (per-core data)
      ONE exp over the slab  [128, 1024] -> PT f16
      for i: OT += V_blk^T @ PT[i]; pacc[i] += PT[i]  (DVE)
    lb = allones^T @ (pacc0+pacc1); ship raw OT and lb row (no normalize)
"""

import numpy as np

B, T, E, H = 4, 4096, 1024, 128
P = 128
NB_E = E // P           # 8 contraction chunks
N_RND = 8               # rounds; round tt = permuted query cols 512tt..+511
SCALE = float(H) ** -0.5
NEG = -30000.0
N_CORES = 8
F32 = np.float32


def _perm_cols(p: int) -> np.ndarray:
    """Permuted token order for core piece p: within each 256-token block the
    own 128 tokens (offset 128p) come first, the other 128 after."""
    out = []
    for g in range(16):
        own = np.arange(256 * g + 128 * p, 256 * g + 128 * p + 128)
        oth = np.arange(256 * g + 128 * (1 - p), 256 * g + 128 * (1 - p) + 128)
        out.append(own)
        out.append(oth)
    return np.concatenate(out)


def _mask_pair(p: int) -> np.ndarray:
    """maskP [128, 2, 512] f16 for the diagonal pair of any round.

    Partition kk = key within own block i (i=0,1); col r = permuted in-tile
    query. Own block i holds absolute keys (512tt +) 256i + 128p + kk; the
    permuted query col r is absolute offset off(r) = 256*(r//256) +
    (128p if (r//128)%2==0 else 128*(1-p)) + r%128. Visible iff
    off(r) >= 256i + 128p + kk.
    """
    kk = np.arange(128)[:, None, None]
    i = np.arange(2)[None, :, None]
    r = np.arange(512)[None, None, :]
    s = r // 128
    own = (s % 2) == 0
    off = 256 * (r // 256) + np.where(own, 128 * p, 128 * (1 - p)) + r % 128
    visible = off >= 256 * i + 128 * p + kk
    return np.where(visible, 0.0, NEG).astype(np.float16)


def _emit(tc, aps):
    from concourse import mybir

    nc = tc.nc
    f32 = mybir.dt.float32
    f16 = mybir.dt.float16
    EXP = mybir.ActivationFunctionType.Exp

    from contextlib import ExitStack

    xT, wqkv, maskP, out_o, out_l = aps

    ctx = ExitStack()
    with ctx:
        # ---- pools ----
        consts = ctx.enter_context(tc.tile_pool(name="consts", bufs=1))
        x_pool = ctx.enter_context(tc.tile_pool(name="x", bufs=3))
        qt_pool = ctx.enter_context(tc.tile_pool(name="qt", bufs=2))
        vt_pool = ctx.enter_context(tc.tile_pool(name="vt", bufs=2))
        pt_pool = ctx.enter_context(tc.tile_pool(name="pt", bufs=3))
        pa_pool = ctx.enter_context(tc.tile_pool(name="pa", bufs=4))
        osb_pool = ctx.enter_context(tc.tile_pool(name="osb", bufs=2))
        lr_pool = ctx.enter_context(tc.tile_pool(name="lr", bufs=2))
        # PSUM: score pair slabs 2x2 banks + proj 2 + ot 2 = 8 banks.
        # lb squats in an idle s_ps slot (scores are done when lb runs).
        s_ps = ctx.enter_context(tc.tile_pool(name="sps", bufs=2, space="PSUM"))
        o_ps = ctx.enter_context(tc.tile_pool(name="ops", bufs=2, space="PSUM"))
        p_ps = ctx.enter_context(tc.tile_pool(name="pps", bufs=2, space="PSUM"))

        # ---- persistent SBUF tensors ----
        allones = consts.tile([P, P], f16)
        identity = consts.tile([P, P], f16)
        w_sb = consts.tile([P, NB_E, 3, P], f16)   # [., chunk, (k|v|q), .]
        mask_sb = consts.tile([P, 2, 512], f16)
        kt_all = consts.tile([P, 16, P], f16)
        v_all = consts.tile([P, 16, P], f16)

        nc.gpsimd.memset(allones[:], 1.0)
        from concourse.masks import make_identity
        make_identity(nc, identity[:])

        # ---- x round tiles: [128, chunk c, block b(128), 128] f16 ----
        # own key tokens of the round sit at b = 0 and 2.
        x_tiles = [x_pool.tile([P, NB_E, 4, 128], f16, tag="x", name=f"x_{tt}")
                   for tt in range(N_RND)]

        def dma_x(tt, split=1, eng=None):
            """Load round tt's 512 permuted query cols, chunk-striped across
            engine queues for parallel descriptor issue."""
            engs = eng if eng is not None else [nc.sync]
            cw = NB_E // split
            for g in range(split):
                engs[g % len(engs)].dma_start(
                    x_tiles[tt][:, g * cw:(g + 1) * cw, :, :],
                    xT[g * cw * P:(g + 1) * cw * P,
                       tt * 512:tt * 512 + 512].rearrange(
                        "(c p) (b q) -> p c b q", p=P, b=4),
                )

        # startup-critical order: weights first (gate all projections), the
        # first rounds of x next, striped across the three DMA-capable
        # engine queues (sync/scalar/gpsimd).
        for ch in (0, 4):
            nc.sync.dma_start(
                w_sb[:, ch:ch + 4, :, :],
                wqkv[ch * P:(ch + 4) * P, :, :].rearrange(
                    "(c p) s h -> p c s h", p=P))
        dma_x(0, split=4, eng=[nc.scalar, nc.gpsimd])
        nc.sync.dma_start(
            mask_sb[:, :, :],
            maskP[:, :, :].rearrange("p i q -> p i q"))
        dma_x(1, split=2, eng=[nc.scalar, nc.gpsimd])
        dma_x(2, split=2, eng=[nc.scalar, nc.gpsimd])

        # PE warmup: dummy matmuls ramp the tensor-engine p-state while the
        # first DMAs stream; their results are never read.
        for _ in range(30):
            wp = p_ps.tile([P, P], f32, tag="pps", name="warm")
            nc.tensor.matmul(wp[:], lhsT=allones[:], rhs=allones[:],
                             start=True, stop=True)

        # ---- projection pieces (generators of thunks) ----
        def q_group(tt, qt):
            xt = x_tiles[tt]
            ps = p_ps.tile([P, 512], f32, tag="pps")
            for c in range(NB_E):
                def mm(c=c, ps=ps):
                    nc.tensor.matmul(ps[:], lhsT=w_sb[:, c, 2, :],
                                     rhs=xt[:, c, :, :], start=(c == 0),
                                     stop=(c == NB_E - 1))
                yield mm
            # Q fin on ACT: keeps the DVE queue (mask/pacc critical ops) short
            yield lambda ps=ps: nc.scalar.copy(qt[:], ps[:])

        def k_group(tt):
            xt = x_tiles[tt]
            ps = p_ps.tile([P, 256], f32, tag="pps", name=f"kps_{tt}")
            for c in range(NB_E):
                def mm(c=c, ps=ps):
                    nc.tensor.matmul(ps[:], lhsT=w_sb[:, c, 0, :],
                                     rhs=xt[:, c, 0::2, :], start=(c == 0),
                                     stop=(c == NB_E - 1))
                yield mm

            def fin(ps=ps):
                nc.vector.tensor_copy(
                    kt_all[:, 2 * tt:2 * tt + 2, :],
                    ps[:].rearrange("p (i q) -> p i q", i=2))
            yield fin

        def v_group(tt):
            xt = x_tiles[tt]
            vt = vt_pool.tile([P, 256], f16, tag="vt", name=f"vt_{tt}")
            ps = p_ps.tile([P, 256], f32, tag="pps", name=f"vps_{tt}")
            for c in range(NB_E):
                def mm(c=c, ps=ps):
                    nc.tensor.matmul(ps[:], lhsT=w_sb[:, c, 1, :],
                                     rhs=xt[:, c, 0::2, :], start=(c == 0),
                                     stop=(c == NB_E - 1))
                yield mm
            yield lambda vt=vt, ps=ps: nc.vector.tensor_copy(vt[:], ps[:])
            for u in range(2):
                kb = 2 * tt + u

                def tr(u=u, kb=kb, vt=vt):
                    tp = p_ps.tile([P, P], f16, tag="pps", name=f"tp_{kb}")
                    nc.tensor.transpose(tp[:], vt[:, u * P:(u + 1) * P],
                                        identity[:])
                    nc.vector.tensor_copy(v_all[:, kb, :], tp[:])
                yield tr

        def chain(*gens):
            for g in gens:
                yield from g

        def drain(gen, n):
            if gen is None:
                return False
            for _ in range(n):
                try:
                    next(gen)()
                except StopIteration:
                    return False
            return True

        qts = [qt_pool.tile([P, 512], f16, tag="qt", name=f"qt_{t}")
               for t in range(N_RND)]

        # round 0 projections run up front
        for piece in chain(q_group(0, qts[0]), k_group(0), v_group(0)):
            piece()

        N_GEN = 9 + 9 + 11   # q + k + v piece counts per round
        gen = None

        for tt in range(N_RND):
            if 1 <= tt and tt + 2 < N_RND:
                dma_x(tt + 2, split=2, eng=[nc.scalar, nc.gpsimd])
            gen = (chain(q_group(tt + 1, qts[tt + 1]), k_group(tt + 1),
                         v_group(tt + 1))
                   if tt + 1 < N_RND else None)
            npair = tt + 1
            rate = -(-N_GEN // npair)

            qs = qts[tt]
            ot = o_ps.tile([P, 512], f32, tag="ops", name=f"ot_{tt}")
            pacc = pa_pool.tile([P, 2, 512], f16, tag="pa", name=f"pa_{tt}")
            nc.gpsimd.memset(pacc[:], 0.0)

            s_tiles = [None] * npair

            def emit_scores(g, tt=tt, qs=qs, s_tiles=s_tiles):
                """Score pair g: own blocks 2g, 2g+1 into one 2-bank slab."""
                s = s_ps.tile([P, 2, 512], f32, tag="sps",
                              name=f"s_{tt}_{g}")
                for i in range(2):
                    c0 = 256 * i if g == tt else 0
                    nc.tensor.matmul(
                        s[:, i, c0:512],
                        lhsT=kt_all[:, 2 * g + i, :],
                        rhs=qs[:, c0:512],
                        start=True, stop=True,
                    )
                s_tiles[g] = s

            emit_scores(0)
            for g in range(npair):
                if g + 1 < npair:
                    emit_scores(g + 1)
                s = s_tiles[g]
                if g == tt:  # diagonal pair: causal boundary mask strips
                    nc.vector.tensor_add(s[:, 0, 0:256], s[:, 0, 0:256],
                                         mask_sb[:, 0, 0:256])
                    nc.vector.tensor_add(s[:, 1, 256:512], s[:, 1, 256:512],
                                         mask_sb[:, 1, 256:512])
                pt = pt_pool.tile([P, 2, 512], f16, tag="pt")
                if g == tt:
                    # diagonal pair: slot 1 cols [0,256) were never written
                    # (causally dead); exp each slot's live range separately
                    nc.scalar.activation(pt[:, 0, :], s[:, 0, :],
                                         EXP, scale=SCALE)
                    nc.scalar.activation(pt[:, 1, 256:512], s[:, 1, 256:512],
                                         EXP, scale=SCALE)
                else:
                    nc.scalar.activation(
                        pt[:].rearrange("p i q -> p (i q)"),
                        s[:].rearrange("p i q -> p (i q)"),
                        EXP, scale=SCALE)
                for i in range(2):
                    c0 = 256 * i if g == tt else 0
                    nc.tensor.matmul(
                        ot[:, c0:512],
                        lhsT=v_all[:, 2 * g + i, :],
                        rhs=pt[:, i, c0:512],
                        start=(g == 0 and i == 0),
                        stop=(g == tt and i == 1),
                    )
                if g == tt:
                    # diagonal: trim per slot (slot 1 cols [0,256) hold
                    # exp(garbage) from the untouched psum region)
                    nc.vector.tensor_add(pacc[:, 0, 0:512], pacc[:, 0, 0:512],
                                         pt[:, 0, 0:512])
                    nc.vector.tensor_add(pacc[:, 1, 256:512],
                                         pacc[:, 1, 256:512],
                                         pt[:, 1, 256:512])
                else:
                    # one fused 3D add for the whole pair
                    nc.vector.tensor_add(pacc[:], pacc[:], pt[:])

                if gen is not None and not drain(gen, rate):
                    gen = None

            while drain(gen, 4):
                pass
            gen = None

            # epilogue: denominator + ship RAW numerator and denominator
            # (host merges the two key-halves). lb reuses an idle score slot.
            lb = s_ps.tile([P, 512], f32, tag="sps", name=f"lb_{tt}")
            halves = (0, 256) if tt == N_RND - 1 else (0,)
            width = 512 // len(halves)
            for hb in halves:
                sl = slice(hb, hb + width)
                nc.tensor.matmul(lb[:, sl], lhsT=allones[:],
                                 rhs=pacc[:, 0, sl], start=True, stop=False)
                nc.tensor.matmul(lb[:, sl], lhsT=allones[:],
                                 rhs=pacc[:, 1, sl], start=False, stop=True)
                o_sb = osb_pool.tile([P, width], f32, tag="osb")
                nc.vector.tensor_copy(o_sb[:], ot[:, sl])
                l_sb = lr_pool.tile([1, width], f32, tag="lr")
                nc.vector.tensor_copy(l_sb[:], lb[0:1, sl])
                nc.gpsimd.dma_start(
                    out_o[:, tt * 512 + hb: tt * 512 + hb + width], o_sb[:])
                nc.gpsimd.dma_start(
                    out_l[:, tt * 512 + hb: tt * 512 + hb + width], l_sb[:])


def build_program():
    import concourse.tile as tile
    from concourse import bacc, mybir

    f32 = mybir.dt.float32
    f16 = mybir.dt.float16
    nc = bacc.Bacc("TRN2", target_bir_lowering=False, debug=False,
                   num_devices=N_CORES)
    xT = nc.dram_tensor("xT", [E, T], f16, kind="ExternalInput").ap()
    wqkv = nc.dram_tensor("wqkv", [E, 3, H], f16, kind="ExternalInput").ap()
    maskP = nc.dram_tensor("maskP", [128, 2, 512], f16,
                           kind="ExternalInput").ap()
    out_o = nc.dram_tensor("out_o", [H, T], f32, kind="ExternalOutput").ap()
    out_l = nc.dram_tensor("out_l", [1, T], f32, kind="ExternalOutput").ap()

    with tile.TileContext(nc) as tc:
        _emit(tc, (xT, wqkv, maskP, out_o, out_l))
    nc.compile()
    return nc


def make_in_maps(x, Wq, Wk, Wv):
    """Per-core input maps. x: [B,T,E] f32; W*: [H,E] f32."""
    x = np.asarray(x, dtype=F32)
    wqkv = np.stack(
        [np.asarray(Wk, F32).T, np.asarray(Wv, F32).T, np.asarray(Wq, F32).T],
        axis=1).astype(np.float16)
    wqkv = np.ascontiguousarray(wqkv)
    masks = [_mask_pair(0), _mask_pair(1)]
    perms = [_perm_cols(0), _perm_cols(1)]
    in_maps = []
    for c in range(N_CORES):
        b, p = c // 2, c % 2
        xb = x[b][perms[p]]                                    # [T, E] permuted
        xT_np = np.ascontiguousarray(xb.T.astype(np.float16))
        in_maps.append({
            "xT": xT_np,
            "wqkv": wqkv,
            "maskP": masks[p],
        })
    return in_maps


def run(x, Wq, Wk, Wv, trace=False, trace_cores=None):
    """Returns (full_output [B,T,H] f32, BassKernelResults)."""
    from concourse.bass_utils import run_bass_kernel_spmd

    nc = build_program()
    in_maps = make_in_maps(x, Wq, Wk, Wv)
    res = run_bass_kernel_spmd(
        nc, in_maps, list(range(N_CORES)), trace=trace,
        trace_cores=trace_cores,
    )
    perms = [_perm_cols(0), _perm_cols(1)]
    full = np.empty((B, T, H), dtype=F32)
    for b in range(B):
        o_sum = np.zeros((H, T), dtype=np.float64)
        l_sum = np.zeros(T, dtype=np.float64)
        for p in range(2):
            r = res.results[2 * b + p]
            o_nat = np.empty((H, T), dtype=np.float64)
            l_nat = np.empty(T, dtype=np.float64)
            o_nat[:, perms[p]] = r["out_o"].astype(np.float64)
            l_nat[perms[p]] = r["out_l"][0].astype(np.float64)
            o_sum += o_nat
            l_sum += l_nat
        full[b] = (o_sum / l_sum).T.astype(F32)
    return full, res


def kernel(x, Wq, Wk, Wv):
    full, _ = run(x, Wq, Wk, Wv, trace=False)
    return full


if __name__ == "__main__":
    nc = build_program()
    print("program built ok")
